# revision 7
# baseline (speedup 1.0000x reference)
"""Trainium2 Bass kernel for nn_LocalInferenceModel_2740189134870.

ESIM-style cross-attention block:
    e   = a @ b^T                       [B, La, Lb]
    t_a = softmax(e, axis=Lb) @ b       [B, La, D]
    t_b = softmax(e, axis=La)^T @ a     [B, Lb, D]
    m_a = concat(a, t_a, a - t_a, a * t_a)
    m_b = concat(b, t_b, b - t_b, b * t_b)

Sharding: data-parallel over batch B=64 across 8 NeuronCores (8 examples
per core). No collectives needed.

Per-example on-core schedule (L=512, D=768, P=128):
  1. Load a, b in natural layout ([128, 4, 768] tiles).
  2. PE-transpose 128x128 blocks to build aT, bT ([128, 6, 512], fp32r).
  3. E chunks [128, 512] = aT^T @ bT in PSUM (fp32r matmuls, fp32 accum).
     Row maxes per chunk -> GPSIMD partition-all-reduce -> global max C.
  4. exp(E - C + 44) straight from PSUM (ACT) into fp32r prob tiles;
     accum_out gives the row sums S_a for free. One global offset per
     example keeps both attention directions consistent without any
     per-row rescale corrections; the +44 keeps the worst-deficit row
     sums (down to ~e^-93 relative to the max) well inside fp32 normals
     while the largest summands (~e^44) stay far from overflow.
  5. PE-transpose the prob chunks to get expET (= exp(E^T - C + 44));
     the PSUM->SBUF copy's accum_out yields S_b for free.
  6. t_b = expE^T @ a_r, t_a = expET^T @ b_r accumulated over 128-chunks;
     normalization 1/S folded into the PSUM->SBUF copy as a per-partition
     ACT scale.
  7. DVE computes a-t_a / a*t_a; 16 DMA stores assemble the concat.
"""

import os
import sys

for _p in ("/opt/trn_rl_repo", "/root/.axon_site/_ro/trn_rl_repo"):
    if os.path.isdir(_p) and _p not in sys.path:
        sys.path.append(_p)

import numpy as np

B, L, D = 64, 512, 768
NCORES = 8
BSH = B // NCORES          # examples per core
P = 128                    # partitions
MCH = L // P               # 4 row chunks
KCH = D // P               # 6 contraction chunks
DS = 384                   # D split for t matmuls (2 PSUM groups)
NSPL = D // DS
EXP_OFF = 44.0             # exp rescale: exp(e - C + 44)

_CACHE = {}


def _build_nc():
    import concourse.bass as bass
    import concourse.bass_isa as bass_isa
    import concourse.mybir as mybir
    import concourse.tile as tile
    from concourse import bacc
    from concourse.masks import make_identity

    f32 = mybir.dt.float32
    f32r = mybir.dt.float32r
    AX = mybir.AxisListType.X
    EXP = mybir.ActivationFunctionType.Exp
    COPY = mybir.ActivationFunctionType.Copy
    IDENT = mybir.ActivationFunctionType.Identity

    nc = bacc.Bacc()
    a_h = nc.declare_dram_parameter("a", [BSH, L, D], f32, isOutput=False)
    b_h = nc.declare_dram_parameter("b", [BSH, L, D], f32, isOutput=False)
    ma_h = nc.declare_dram_parameter("ma", [BSH, L, 4 * D], f32, isOutput=True)
    mb_h = nc.declare_dram_parameter("mb", [BSH, L, 4 * D], f32, isOutput=True)

    with tile.TileContext(nc) as tc:
        with tc.tile_pool(name="const", bufs=1) as const_pool, \
             tc.tile_pool(name="io", bufs=2) as io_pool, \
             tc.tile_pool(name="tp", bufs=1) as tp_pool, \
             tc.tile_pool(name="esb", bufs=2) as e_pool, \
             tc.tile_pool(name="rsb", bufs=1) as r_pool, \
             tc.tile_pool(name="tsb", bufs=2) as t_pool, \
             tc.tile_pool(name="cmb", bufs=3) as c_pool, \
             tc.tile_pool(name="st", bufs=2) as s_pool, \
             tc.tile_pool(name="ps", bufs=2, space="PSUM") as tr_ps, \
             tc.tile_pool(name="pe", bufs=4, space="PSUM") as e_ps, \
             tc.tile_pool(name="pt", bufs=2, space="PSUM") as t_ps:

            ident = const_pool.tile([P, P], f32)
            make_identity(nc, ident)
            identr = const_pool.tile([P, P], f32r)
            nc.scalar.copy(out=identr, in_=ident)

            for x in range(BSH):
                # ---- load natural layouts -------------------------------
                a_nat = io_pool.tile([P, MCH, D], f32, tag="anat")
                b_nat = io_pool.tile([P, MCH, D], f32, tag="bnat")
                nc.sync.dma_start(
                    out=a_nat, in_=a_h[x].rearrange("(m p) d -> p m d", p=P))
                nc.sync.dma_start(
                    out=b_nat, in_=b_h[x].rearrange("(m p) d -> p m d", p=P))

                # ---- build transposed (D-major) copies on PE ------------
                aT = tp_pool.tile([P, KCH, L], f32r, tag="aT")
                bT = tp_pool.tile([P, KCH, L], f32r, tag="bT")
                for src, dst in ((a_nat, aT), (b_nat, bT)):
                    for k in range(KCH):
                        ps = tr_ps.tile([P, L], f32, tag="tr")
                        for m in range(MCH):
                            nc.tensor.transpose(
                                ps[:, m * P:(m + 1) * P],
                                src[:, m, k * P:(k + 1) * P],
                                ident)
                        nc.scalar.copy(out=dst[:, k, :], in_=ps)

                # ---- E chunks (held in PSUM), row maxes -----------------
                eps_chunks = []
                uv = s_pool.tile([P, MCH], f32, tag="uv")
                for m in range(MCH):
                    ps = e_ps.tile([P, L], f32, tag="e")
                    for k in range(KCH):
                        nc.tensor.matmul(
                            ps,
                            aT[:, k, m * P:(m + 1) * P],
                            bT[:, k, :],
                            start=(k == 0), stop=(k == KCH - 1))
                    nc.vector.reduce_max(
                        out=uv[:, m:m + 1], in_=ps, axis=AX)
                    eps_chunks.append(ps)

                # ---- global max C -> bias (44 - C) broadcast ------------
                m4 = s_pool.tile([P, 1], f32, tag="m4")
                nc.vector.reduce_max(out=m4, in_=uv, axis=AX)
                mall = s_pool.tile([P, 1], f32, tag="mall")
                nc.gpsimd.partition_all_reduce(
                    mall, m4, channels=P, reduce_op=bass_isa.ReduceOp.max)
                cneg = s_pool.tile([P, 1], f32, tag="cneg")
                nc.vector.tensor_scalar(
                    out=cneg, in0=mall, scalar1=-1.0, scalar2=EXP_OFF,
                    op0=mybir.AluOpType.mult, op1=mybir.AluOpType.add)

                # ---- exp from PSUM + row sums S_a -----------------------
                expE = e_pool.tile([P, MCH, L], f32r, tag="expE")
                sa = s_pool.tile([P, MCH], f32, tag="sa")
                for m in range(MCH):
                    nc.scalar.activation(
                        out=expE[:, m, :], in_=eps_chunks[m],
                        func=EXP, bias=cneg, scale=1.0,
                        accum_out=sa[:, m:m + 1])

                # ---- transpose probs -> expET, col sums S_b -------------
                expET = e_pool.tile([P, MCH, L], f32r, tag="expET")
                sb = s_pool.tile([P, MCH], f32, tag="sb")
                for n in range(MCH):
                    ps = tr_ps.tile([P, L], f32r, tag="tr")
                    for m in range(MCH):
                        nc.tensor.transpose(
                            ps[:, m * P:(m + 1) * P],
                            expE[:, m, n * P:(n + 1) * P],
                            identr)
                    nc.scalar.activation(
                        out=expET[:, n, :], in_=ps,
                        func=COPY, accum_out=sb[:, n:n + 1])

                rsa = s_pool.tile([P, MCH], f32, tag="rsa")
                rsb = s_pool.tile([P, MCH], f32, tag="rsb")
                nc.vector.reciprocal(out=rsa, in_=sa)
                nc.vector.reciprocal(out=rsb, in_=sb)

                # ---- fp32r-rounded copies for the t-matmul rhs ----------
                a_r = r_pool.tile([P, MCH, D], f32r, tag="ar")
                b_r = r_pool.tile([P, MCH, D], f32r, tag="br")
                nc.vector.tensor_copy(out=a_r, in_=a_nat)
                nc.vector.tensor_copy(out=b_r, in_=b_nat)

                # ---- t_b and t_a ---------------------------------------
                tb = t_pool.tile([P, MCH, D], f32, tag="tb")
                ta = t_pool.tile([P, MCH, D], f32, tag="ta")
                for lt, rt, dst, rs in (
                        (expE, a_r, tb, rsb), (expET, b_r, ta, rsa)):
                    for n in range(MCH):
                        for c in range(NSPL):
                            ps = t_ps.tile([P, DS], f32, tag="t")
                            for m in range(MCH):
                                nc.tensor.matmul(
                                    ps,
                                    lt[:, m, n * P:(n + 1) * P],
                                    rt[:, m, c * DS:(c + 1) * DS],
                                    start=(m == 0), stop=(m == MCH - 1))
                            nc.scalar.activation(
                                out=dst[:, n, c * DS:(c + 1) * DS],
                                in_=ps, func=COPY,
                                scale=rs[:, n:n + 1])

                # ---- combos + stores -----------------------------------
                for nat, t, out_h in ((a_nat, ta, ma_h), (b_nat, tb, mb_h)):
                    for m in range(MCH):
                        rows = slice(m * P, (m + 1) * P)
                        dif = c_pool.tile([P, D], f32, tag="dif")
                        prd = c_pool.tile([P, D], f32, tag="prd")
                        nc.vector.tensor_sub(dif, nat[:, m, :], t[:, m, :])
                        nc.vector.tensor_mul(prd, nat[:, m, :], t[:, m, :])
                        nc.sync.dma_start(
                            out=out_h[x, rows, 0:D], in_=nat[:, m, :])
                        nc.sync.dma_start(
                            out=out_h[x, rows, D:2 * D], in_=t[:, m, :])
                        nc.sync.dma_start(
                            out=out_h[x, rows, 2 * D:3 * D], in_=dif)
                        nc.sync.dma_start(
                            out=out_h[x, rows, 3 * D:4 * D], in_=prd)

    nc.finalize()
    return nc


def _get_nc():
    if "nc" not in _CACHE:
        _CACHE["nc"] = _build_nc()
    return _CACHE["nc"]


def _numpy_fallback(a, mask_a, b, mask_b):
    NEG = -100000.0
    e = np.einsum("bid,bjd->bij", a, b)
    mask_e = mask_a[:, :, None].astype(np.float32) * \
        mask_b[:, None, :].astype(np.float32)
    e = np.where(mask_e < 0.5, NEG, e)

    def softmax(x, axis):
        x = x - x.max(axis=axis, keepdims=True)
        ex = np.exp(x)
        return ex / ex.sum(axis=axis, keepdims=True)

    t_a = np.einsum("bij,bjd->bid", softmax(e, 2), b)
    t_b = np.einsum("bij,bid->bjd", softmax(e, 1), a)
    m_a = np.concatenate((a, t_a, a - t_a, a * t_a), axis=-1)
    m_b = np.concatenate((b, t_b, b - t_b, b * t_b), axis=-1)
    return m_a, m_b


def kernel(a, mask_a, b, mask_b):
    a = np.ascontiguousarray(np.asarray(a, dtype=np.float32))
    b = np.ascontiguousarray(np.asarray(b, dtype=np.float32))
    mask_a = np.asarray(mask_a)
    mask_b = np.asarray(mask_b)

    if not (np.all(mask_a == 1) and np.all(mask_b == 1)):
        return _numpy_fallback(a, mask_a, b, mask_b)

    from concourse.bass_utils import run_bass_kernel_spmd

    nc = _get_nc()
    in_maps = [
        {"a": a[i * BSH:(i + 1) * BSH], "b": b[i * BSH:(i + 1) * BSH]}
        for i in range(NCORES)
    ]
    res = run_bass_kernel_spmd(nc, in_maps, core_ids=list(range(NCORES))).results
    m_a = np.concatenate([r["ma"] for r in res], axis=0)
    m_b = np.concatenate([r["mb"] for r in res], axis=0)
    return m_a, m_b
